# revision 1
# baseline (speedup 1.0000x reference)
"""Trainium2 Bass kernel for the ClassifierModel IoU-match loss.

Strategy: data-parallel over the batch axis B across 8 NeuronCores
(16 images per core). Inside each core the per-image [L=128, P=4096]
IoU/argmax matching is computed in 32 chunks of [128 proposals x 128
labels], fully fused on-chip (no [B,L,P] materialization in HBM):

  - proposals live on SBUF partitions, labels along the free axis
  - label coordinate rows are broadcast to [128,128] tiles via K=1
    ones-matmuls on the (otherwise idle) TensorEngine
  - intersection width/height via fused tensor_scalar dual-op and
    tensor_tensor instructions
  - iou ordering uses the proxy inter/(area_l + area_r), which is a
    strictly monotone transform of inter/union per proposal, computed
    with a fused tensor_tensor_reduce (divide + running max)
  - the argmax one-hot mask is built by an is_equal compare against the
    row max, transposed on the TensorEngine and used as matmul weights
    to gather matched label coords + label index in one shot
  - bbox targets, Huber, and the softmax cross-entropy terms are
    evaluated on wide [128, 512] tiles covering all 16 images at once,
    reduced to a single scalar per core; host sums the 8 core scalars.
"""

import sys

import numpy as np

sys.path.insert(0, "/opt/trn_rl_repo")

B, P, L = 128, 4096, 128
NCORES = 8
IMG = B // NCORES            # images per core
C = P // 128                 # chunks (free columns) per image
SCALE = 32.0
USE_DIV = True               # tensor_tensor_reduce with AluOp.divide

LOG01 = float(np.log(np.float32(0.1)))
LOG09 = float(np.log(np.float32(0.9)))
CE_SLOPE = LOG01 - LOG09     # ~ -2.1972246


def build(img=IMG, use_div=USE_DIV):
    """Build + compile the per-core Bass program. Returns the Bacc."""
    from contextlib import ExitStack

    import concourse.tile as tile
    from concourse import bacc, mybir

    f32 = mybir.dt.float32
    Alu = mybir.AluOpType
    Act = mybir.ActivationFunctionType
    X = mybir.AxisListType.X

    W = img * C              # wide column count (img=16 -> 512)
    nc = bacc.Bacc("TRN2", target_bir_lowering=False, debug=False,
                   enable_asserts=True, num_devices=NCORES)

    cls_d = nc.dram_tensor("cls", [img, 2 * P], f32, kind="ExternalInput").ap()
    bbox_d = nc.dram_tensor("bbox", [img, 4 * P], f32, kind="ExternalInput").ap()
    roi_d = nc.dram_tensor("roi", [img, P, 4], f32, kind="ExternalInput").ap()
    lab_d = nc.dram_tensor("labels", [img, L, 4], f32, kind="ExternalInput").ap()
    ident_d = nc.dram_tensor("ident", [128, 128], f32, kind="ExternalInput").ap()
    iotap_d = nc.dram_tensor("iotap", [128, 1], f32, kind="ExternalInput").ap()
    ngate_d = nc.dram_tensor("ngate", [128, 1], f32, kind="ExternalInput").ap()
    onesr_d = nc.dram_tensor("onesr", [1, 128], f32, kind="ExternalInput").ap()
    onesc_d = nc.dram_tensor("onesc", [128, 1], f32, kind="ExternalInput").ap()
    out_d = nc.dram_tensor("out", [1, 1], f32, kind="ExternalOutput").ap()

    with tile.TileContext(nc) as tc, ExitStack() as ctx:
        cpool = ctx.enter_context(tc.tile_pool(name="consts", bufs=1))
        wpool = ctx.enter_context(tc.tile_pool(name="wide", bufs=1))
        ipool = ctx.enter_context(tc.tile_pool(name="perimg", bufs=2))
        kpool = ctx.enter_context(tc.tile_pool(name="chunk", bufs=4))
        pT = ctx.enter_context(tc.tile_pool(name="ptrans", bufs=2, space="PSUM"))
        pM = ctx.enter_context(tc.tile_pool(name="pmatch", bufs=2, space="PSUM"))
        pB = ctx.enter_context(tc.tile_pool(name="pbcast", bufs=1, space="PSUM"))
        pS = ctx.enter_context(tc.tile_pool(name="psmall", bufs=1, space="PSUM"))

        # ---- constants
        ident = cpool.tile([128, 128], f32)
        nc.sync.dma_start(ident[:], ident_d[:])
        iotap = cpool.tile([128, 1], f32)
        nc.sync.dma_start(iotap[:], iotap_d[:])
        ngate = cpool.tile([128, 1], f32)
        nc.sync.dma_start(ngate[:], ngate_d[:])
        onesr = cpool.tile([1, 128], f32)
        nc.sync.dma_start(onesr[:], onesr_d[:])
        onesc = cpool.tile([128, 1], f32)
        nc.sync.dma_start(onesc[:], onesc_d[:])

        # ---- per-core wide tiles (col j = i*C + c; proposal g = p*C + c)
        CLS0 = wpool.tile([128, W], f32)
        CLS1 = wpool.tile([128, W], f32)
        PRED = wpool.tile([128, 4 * W], f32)   # col = k*W + j
        RXS = wpool.tile([128, W], f32)
        RYS = wpool.tile([128, W], f32)
        RWS = wpool.tile([128, W], f32)
        RHS_ = wpool.tile([128, W], f32)
        RA = wpool.tile([128, W], f32)
        RXW = wpool.tile([128, W], f32)
        RYH = wpool.tile([128, W], f32)
        MX = wpool.tile([128, W], f32)
        MATCH = wpool.tile([128, 5 * W], f32)  # col = j*5 + k

        for i in range(img):
            jsl = slice(i * C, (i + 1) * C)

            # ---- loads (contiguous rows per partition)
            ROI = ipool.tile([128, 4 * C], f32, tag="roi")   # (c,k) interleaved
            nc.sync.dma_start(
                ROI[:], roi_d[i].rearrange("(p c) k -> p (c k)", c=C))
            LAB = ipool.tile([128, 4], f32, tag="lab")
            nc.sync.dma_start(LAB[:], lab_d[i])
            nc.sync.dma_start(
                CLS0[:, jsl], cls_d[i, 0:P].rearrange("(p c) -> p c", c=C))
            nc.sync.dma_start(
                CLS1[:, jsl], cls_d[i, P:2 * P].rearrange("(p c) -> p c", c=C))
            for k in range(4):
                nc.sync.dma_start(
                    PRED[:, k * W + i * C: k * W + (i + 1) * C],
                    bbox_d[i, k * P:(k + 1) * P].rearrange("(p c) -> p c", c=C))

            # ---- per-proposal scalars (scale feature->image coords)
            rv = ROI[:].rearrange("p (c k) -> p k c", k=4)
            nc.scalar.activation(RXS[:, jsl], rv[:, 0], Act.Copy, scale=SCALE)
            nc.scalar.activation(RYS[:, jsl], rv[:, 1], Act.Copy, scale=SCALE)
            nc.scalar.activation(RWS[:, jsl], rv[:, 2], Act.Copy, scale=SCALE)
            nc.scalar.activation(RHS_[:, jsl], rv[:, 3], Act.Copy, scale=SCALE)
            nc.vector.tensor_tensor(RA[:, jsl], RWS[:, jsl], RHS_[:, jsl],
                                    Alu.mult)
            nc.vector.tensor_tensor(RXW[:, jsl], RXS[:, jsl], RWS[:, jsl],
                                    Alu.add)
            nc.vector.tensor_tensor(RYH[:, jsl], RYS[:, jsl], RHS_[:, jsl],
                                    Alu.add)

            # ---- label rows -> broadcast tiles [128, 640]
            # single-partition rows [lx | ly | lw | lh | la] so matmul rhs
            # slices sit at base partition 0; loaded transposed from DRAM
            LROW = ipool.tile([1, 640], f32, tag="lrow")
            nc.sync.dma_start(
                LROW[0:1, 0:512].rearrange("p (k l) -> p k l", k=4),
                lab_d[i].rearrange("l k -> k l")[None])
            nc.vector.tensor_tensor(LROW[0:1, 512:640], LROW[0:1, 256:384],
                                    LROW[0:1, 384:512], Alu.mult)  # la = lw*lh
            lxr = LROW[0:1, 0:128]
            lyr = LROW[0:1, 128:256]
            lwr = LROW[0:1, 256:384]
            lhr = LROW[0:1, 384:512]
            lar = LROW[0:1, 512:640]
            BCp = pB.tile([128, 640], f32, tag="bcp")
            # blocks: LX, LXW, LY, LYH, LA
            nc.tensor.matmul(BCp[:, 0:128], onesr[:], lxr, start=True, stop=True)
            nc.tensor.matmul(BCp[:, 128:256], onesr[:], lxr,
                             start=True, stop=False)
            nc.tensor.matmul(BCp[:, 128:256], onesr[:], lwr,
                             start=False, stop=True)
            nc.tensor.matmul(BCp[:, 256:384], onesr[:], lyr,
                             start=True, stop=True)
            nc.tensor.matmul(BCp[:, 384:512], onesr[:], lyr,
                             start=True, stop=False)
            nc.tensor.matmul(BCp[:, 384:512], onesr[:], lhr,
                             start=False, stop=True)
            nc.tensor.matmul(BCp[:, 512:640], onesr[:], lar,
                             start=True, stop=True)
            BC = ipool.tile([128, 640], f32, tag="bc")
            nc.scalar.copy(BC[:], BCp[:])
            LX, LXW = BC[:, 0:128], BC[:, 128:256]
            LY, LYH = BC[:, 256:384], BC[:, 384:512]
            LA = BC[:, 512:640]

            # ---- gather rhs: [lx, ly, lw, lh, label_index]
            LAB5 = ipool.tile([128, 5], f32, tag="lab5")
            nc.scalar.copy(LAB5[:, 0:4], LAB[:])
            nc.scalar.copy(LAB5[:, 4:5], iotap[:])

            MPS = pM.tile([128, 5 * C], f32, tag="mps")
            for c in range(C):
                j = i * C + c
                rx = RXS[:, j:j + 1]
                ry = RYS[:, j:j + 1]
                rxw = RXW[:, j:j + 1]
                ryh = RYH[:, j:j + 1]
                ra = RA[:, j:j + 1]

                # ix = min(lxw, rxw) - max(lx, rx); iy likewise
                m2 = kpool.tile([128, 128], f32, tag="m2")
                nc.gpsimd.tensor_scalar(m2[:], LX, rx, None, Alu.max)
                u = kpool.tile([128, 128], f32, tag="u")
                nc.vector.scalar_tensor_tensor(u[:], LXW, rxw, m2[:],
                                               Alu.min, Alu.subtract)
                m4 = kpool.tile([128, 128], f32, tag="m4")
                nc.gpsimd.tensor_scalar(m4[:], LY, ry, None, Alu.max)
                v = kpool.tile([128, 128], f32, tag="v")
                nc.vector.scalar_tensor_tensor(v[:], LYH, ryh, m4[:],
                                               Alu.min, Alu.subtract)
                ur = kpool.tile([128, 128], f32, tag="ur")
                nc.scalar.activation(ur[:], u[:], Act.Relu)
                inter = kpool.tile([128, 128], f32, tag="inter")
                nc.vector.tensor_tensor(inter[:], ur[:], v[:], Alu.mult)
                # S = la + ra > 0, via Relu(la*1 + ra) on ACT
                S = kpool.tile([128, 128], f32, tag="S")
                nc.scalar.activation(S[:], LA, Act.Relu, bias=ra)
                R = kpool.tile([128, 128], f32, tag="R")
                nc.vector.reciprocal_approx_fast(R[:], S[:])
                prox = kpool.tile([128, 128], f32, tag="prox")
                nc.vector.tensor_tensor(prox[:], inter[:], R[:], Alu.mult)
                nc.vector.tensor_reduce(MX[:, j:j + 1], prox[:], X, Alu.max)
                msk = kpool.tile([128, 128], f32, tag="msk")
                nc.gpsimd.tensor_scalar(msk[:], prox[:], MX[:, j:j + 1], None,
                                        Alu.is_equal)
                mskTp = pT.tile([128, 128], f32, tag="mskt")
                nc.tensor.transpose(mskTp[:], msk[:], ident[:])
                mskT = kpool.tile([128, 128], f32, tag="msks")
                nc.scalar.copy(mskT[:], mskTp[:])
                nc.tensor.matmul(MPS[:, c * 5:(c + 1) * 5], mskT[:], LAB5[:],
                                 start=True, stop=True)
            nc.scalar.copy(MATCH[:, i * 5 * C:(i + 1) * 5 * C], MPS[:])

        # ---- wide per-proposal loss stage, col j = (i, c)
        def mview(k):
            return MATCH[:].rearrange("p (j k) -> p k j", k=5)[:, k]

        RCPW = wpool.tile([128, W], f32)
        nc.vector.reciprocal(RCPW[:], RWS[:])
        RCPH = wpool.tile([128, W], f32)
        nc.vector.reciprocal(RCPH[:], RHS_[:])

        pidx = wpool.tile([128, W], f32)
        nc.vector.tensor_scalar(pidx[:], mview(4), 0.5, None, Alu.is_gt)
        hit = wpool.tile([128, W], f32)
        nc.vector.tensor_scalar(hit[:], MX[:], 0.0, None, Alu.is_gt)
        pos = wpool.tile([128, W], f32)
        nc.vector.tensor_tensor(pos[:], hit[:], pidx[:], Alu.mult)

        T4 = wpool.tile([128, 4 * W], f32)
        tmp = wpool.tile([128, W], f32)
        # tx, ty
        nc.vector.tensor_tensor(tmp[:], mview(0), RXS[:], Alu.subtract)
        nc.vector.tensor_tensor(T4[:, 0:W], tmp[:], RCPW[:], Alu.mult)
        tmp2 = wpool.tile([128, W], f32)
        nc.vector.tensor_tensor(tmp2[:], mview(1), RYS[:], Alu.subtract)
        nc.vector.tensor_tensor(T4[:, W:2 * W], tmp2[:], RCPH[:], Alu.mult)
        # tw, th (safe log)
        qw = wpool.tile([128, W], f32)
        nc.vector.tensor_tensor(qw[:], mview(2), RCPW[:], Alu.mult)
        nc.vector.tensor_scalar(qw[:], qw[:], 1e-8, None, Alu.max)
        nc.scalar.activation(T4[:, 2 * W:3 * W], qw[:], Act.Ln)
        qh = wpool.tile([128, W], f32)
        nc.vector.tensor_tensor(qh[:], mview(3), RCPH[:], Alu.mult)
        nc.vector.tensor_scalar(qh[:], qh[:], 1e-8, None, Alu.max)
        nc.scalar.activation(T4[:, 3 * W:4 * W], qh[:], Act.Ln)

        # Huber over the packed [128, 4W] tiles
        ERR = wpool.tile([128, 4 * W], f32)
        nc.vector.tensor_tensor(ERR[:], T4[:], PRED[:], Alu.subtract)
        AE = wpool.tile([128, 4 * W], f32)
        nc.scalar.activation(AE[:], ERR[:], Act.Abs)
        M_ = wpool.tile([128, 4 * W], f32)
        nc.vector.tensor_scalar(M_[:], AE[:], 1.0, None, Alu.min)
        U1 = wpool.tile([128, 4 * W], f32)
        nc.vector.tensor_scalar(U1[:], M_[:], 0.5, -1.0, Alu.mult, Alu.add)
        V1 = wpool.tile([128, 4 * W], f32)
        nc.vector.tensor_tensor(V1[:], U1[:], M_[:], Alu.mult)
        H4 = wpool.tile([128, 4 * W], f32)
        nc.vector.tensor_tensor(H4[:], V1[:], AE[:], Alu.add)
        hs01 = wpool.tile([128, W], f32)
        nc.vector.tensor_tensor(hs01[:], H4[:, 0:W], H4[:, W:2 * W], Alu.add)
        hs23 = wpool.tile([128, W], f32)
        nc.vector.tensor_tensor(hs23[:], H4[:, 2 * W:3 * W], H4[:, 3 * W:4 * W],
                                Alu.add)
        hsum = wpool.tile([128, W], f32)
        nc.vector.tensor_tensor(hsum[:], hs01[:], hs23[:], Alu.add)

        # classification terms from sig = softmax prob of class 1
        dlog = wpool.tile([128, W], f32)
        nc.vector.tensor_tensor(dlog[:], CLS1[:], CLS0[:], Alu.subtract)
        sig = wpool.tile([128, W], f32)
        nc.scalar.activation(sig[:], dlog[:], Act.Sigmoid)
        cepos = wpool.tile([128, W], f32)
        nc.vector.tensor_scalar(cepos[:], sig[:], CE_SLOPE, -LOG01,
                                Alu.mult, Alu.add)
        negt = wpool.tile([128, W], f32)
        nc.vector.tensor_scalar(negt[:], sig[:], -CE_SLOPE, -LOG09,
                                Alu.mult, Alu.add)
        nc.vector.tensor_scalar(negt[:], negt[:], ngate[:, 0:1], None, Alu.mult)

        # per_prop = pos * (0.5*hsum + cepos - negt) + negt
        inner = wpool.tile([128, W], f32)
        nc.vector.tensor_scalar(inner[:], hsum[:], 0.5, None, Alu.mult)
        nc.vector.tensor_tensor(inner[:], inner[:], cepos[:], Alu.add)
        nc.vector.tensor_tensor(inner[:], inner[:], negt[:], Alu.subtract)
        nc.vector.tensor_tensor(inner[:], inner[:], pos[:], Alu.mult)
        nc.vector.tensor_tensor(inner[:], inner[:], negt[:], Alu.add)

        PPR = wpool.tile([128, 1], f32)
        nc.vector.tensor_reduce(PPR[:], inner[:], X, Alu.add)
        FS = pS.tile([1, 1], f32, tag="fs")
        nc.tensor.matmul(FS[:], PPR[:], onesc[:], start=True, stop=True)
        fsb = cpool.tile([1, 1], f32)
        nc.scalar.copy(fsb[:], FS[:])
        nc.sync.dma_start(out_d[:], fsb[:])

    nc.compile()
    return nc


_CACHE = {}


def _get_program(img=IMG, use_div=USE_DIV):
    key = (img, use_div)
    if key not in _CACHE:
        _CACHE[key] = build(img, use_div)
    return _CACHE[key]


def _make_in_maps(np_inputs):
    cls_scores = np.ascontiguousarray(np_inputs["cls_scores"], dtype=np.float32)
    bbox_deltas = np.ascontiguousarray(np_inputs["bbox_deltas"],
                                       dtype=np.float32)
    roi = np.ascontiguousarray(np_inputs["roi"], dtype=np.float32)
    labels = np.ascontiguousarray(np_inputs["labels"], dtype=np.float32)
    ng = np.float32(
        1.0 if int(np.asarray(np_inputs["neg_enabled"])) > 0 else 0.0)

    ident = np.eye(128, dtype=np.float32)
    iotap = np.arange(128, dtype=np.float32).reshape(128, 1)
    ngate = np.full((128, 1), ng, dtype=np.float32)
    onesr = np.ones((1, 128), dtype=np.float32)
    onesc = np.ones((128, 1), dtype=np.float32)

    in_maps = []
    for core in range(NCORES):
        sl = slice(core * IMG, (core + 1) * IMG)
        in_maps.append({
            "cls": cls_scores[sl],
            "bbox": bbox_deltas[sl],
            "roi": roi[sl],
            "labels": labels[sl],
            "ident": ident,
            "iotap": iotap,
            "ngate": ngate,
            "onesr": onesr,
            "onesc": onesc,
        })
    return in_maps


def kernel(cls_scores, bbox_deltas, roi, labels, neg_enabled):
    from concourse.bass_utils import run_bass_kernel_spmd

    nc = _get_program()
    in_maps = _make_in_maps({
        "cls_scores": cls_scores,
        "bbox_deltas": bbox_deltas,
        "roi": roi,
        "labels": labels,
        "neg_enabled": neg_enabled,
    })
    res = run_bass_kernel_spmd(nc, in_maps, list(range(NCORES)))
    total = np.float32(0.0)
    for r in res.results:
        total += np.float32(r["out"][0, 0])
    return np.float32(total)

